# revision 1
# baseline (speedup 1.0000x reference)
"""GNN message-passing kernel for Trainium2 (8 NeuronCores, SPMD).

Strategy (hardcoded for the nn_DoormanAgent problem):
  - 65536 nodes = 64 graphs x 1024; shard 8192 nodes (8 graphs) per core.
  - Activations live transposed in SBUF: [128 HID partitions x nodes free], bf16.
  - Per layer: u_loc = x @ W_aggr (node-major psum tiles) -> DRAM -> AllGather
    into a [65536,128] bf16 table; edges (pre-grouped by dst tile on host) are
    fetched with gpsimd.dma_gather (int16 idx, lo/hi half tables) and
    segment-summed on TensorE via one-hot S matrices built on VectorE
    (is_equal vs iota); degree*b_aggr added via a K=1 outer-product matmul.
  - Final BatchNorm via per-channel partial sums + a tiny AllReduce; last
    matmul 256->2 done per 128-node tile with a K=1 bias trick.
"""

import sys

sys.path.insert(0, "/opt/trn_rl_repo")

import numpy as np
import ml_dtypes

import concourse.bass as bass
import concourse.bacc as bacc
import concourse.mybir as mybir
import concourse.tile as tile
from concourse.bass_utils import run_bass_kernel_spmd
from concourse.library_config import mlp as mlp_library

BF16 = mybir.dt.bfloat16
F32 = mybir.dt.float32
I16 = mybir.dt.int16

N = 65536
E = 524288
NCORES = 8
NPC = N // NCORES            # 8192 nodes per core
TPC = NPC // 128             # 64 dst tiles per core
NPG = 1024                   # nodes per graph
GPC = NPC // NPG             # 8 graphs per core
IN_DIM, HID, OUT_DIM, LAYERS = 64, 128, 2, 3
EPS = 1e-5
HALF = N // 2                # int16 gather index limit split
TILE_BATCH = 4               # dst tiles per gather-call pair

_cache = {}


def _host_prep(ei):
    """Group each core's incident edges by (dst tile, src half); pad counts to
    the cross-core max so the SPMD instruction stream is identical on all
    cores. Returns schedule + per-core device arrays."""
    src = np.asarray(ei[0], dtype=np.int64)
    dst = np.asarray(ei[1], dtype=np.int64)

    # per (core, tile, half): list of (src_local, dst_rel)
    counts = np.zeros((NCORES, TPC, 2), dtype=np.int64)
    groups = [[[None, None] for _ in range(TPC)] for _ in range(NCORES)]
    core_of = dst // NPC
    tile_of = (dst % NPC) // 128
    rel_of = dst % 128
    half_of = (src >= HALF).astype(np.int64)
    for c in range(NCORES):
        mc = core_of == c
        for h in (0, 1):
            m = mc & (half_of == h)
            t_arr = tile_of[m]
            s_arr = src[m] - h * HALF
            r_arr = rel_of[m]
            order = np.argsort(t_arr, kind="stable")
            t_arr, s_arr, r_arr = t_arr[order], s_arr[order], r_arr[order]
            bounds = np.searchsorted(t_arr, np.arange(TPC + 1))
            for t in range(TPC):
                lo, hi = bounds[t], bounds[t + 1]
                groups[c][t][h] = (s_arr[lo:hi], r_arr[lo:hi])
                counts[c, t, h] = hi - lo

    gmax = counts.max(axis=0)                         # [TPC, 2]
    nchunks = -(-gmax // 128)                         # ceil, 0 if empty

    # call layout: per batch of TILE_BATCH tiles, one call per half
    calls = []          # list of dicts: half, tiles, chunk spans, idx col offset
    idx_cols = 0
    chunk_tot = 0
    chunk_index = {}    # (t, h, k) -> global chunk id (dstrel column)
    for b in range(TPC // TILE_BATCH):
        tiles = list(range(b * TILE_BATCH, (b + 1) * TILE_BATCH))
        for h in (0, 1):
            spans = []   # (t, call-chunk offset, nchunks)
            ck = 0
            for t in tiles:
                nk = int(nchunks[t, h])
                if nk == 0:
                    continue
                spans.append((t, ck, nk))
                for k in range(nk):
                    chunk_index[(t, h, k)] = chunk_tot + ck + k
                ck += nk
            calls.append(
                dict(half=h, spans=spans, nchunks=ck,
                     idx_col=idx_cols, chunk_off=chunk_tot)
            )
            idx_cols += ck * 8
            chunk_tot += ck

    sched = dict(nchunks=nchunks, calls=calls, chunk_index=chunk_index,
                 idx_cols=idx_cols, chunk_tot=chunk_tot)

    # per-core arrays
    per_core = []
    for c in range(NCORES):
        idx16 = np.zeros((16, max(idx_cols, 8)), dtype=np.int16)
        dstrel = np.full((128, max(chunk_tot, 1)), -1.0, dtype=np.float32)
        for call in calls:
            h = call["half"]
            for (t, ck0, nk) in call["spans"]:
                s_arr, r_arr = groups[c][t][h]
                cnt = len(s_arr)
                nslots = nk * 128
                sl = np.zeros(nslots, dtype=np.int16)
                sl[:cnt] = s_arr.astype(np.int16)
                rl = np.full(nslots, -1.0, dtype=np.float32)
                rl[:cnt] = r_arr.astype(np.float32)
                # slot i -> idx[(i % 16), base + i // 16]; dst_rel[(i % 128), chunk]
                base = call["idx_col"] + ck0 * 8
                idx16[:, base:base + nk * 8] = sl.reshape(nk * 8, 16).T
                for k in range(nk):
                    dstrel[:, chunk_index[(t, h, k)]] = rl[k * 128:(k + 1) * 128]
        idx = np.tile(idx16, (8, 1))
        deg = np.bincount(dst[core_of == c] % NPC, minlength=NPC).astype(np.float32)
        per_core.append(dict(
            idx=idx,
            dstrel=dstrel,
            deg=deg.reshape(1, NPC).astype(ml_dtypes.bfloat16),
        ))
    return sched, per_core


def _build_nc(sched, skip_gather=False, skip_ag=False, nlayers=LAYERS):
    nc = bacc.Bacc("TRN2", target_bir_lowering=False, debug=False)

    # ---- dram parameters (inputs) ----
    p_xT0 = nc.declare_dram_parameter("xT0", [IN_DIM, NPC], F32, isOutput=False)
    p_idx = nc.declare_dram_parameter("idx", [128, max(sched["idx_cols"], 8)], I16, isOutput=False)
    p_dstrel = nc.declare_dram_parameter("dstrel", [128, max(sched["chunk_tot"], 1)], F32, isOutput=False)
    p_deg = nc.declare_dram_parameter("deg", [1, NPC], BF16, isOutput=False)
    p_iota = nc.declare_dram_parameter("iota", [128, 128], BF16, isOutput=False)
    p_wproj = nc.declare_dram_parameter("wproj", [IN_DIM, HID], F32, isOutput=False)
    p_wl = nc.declare_dram_parameter("wl", [HID, LAYERS * HID], F32, isOutput=False)
    p_wa = nc.declare_dram_parameter("wa", [HID, LAYERS * HID], F32, isOutput=False)
    p_bag = nc.declare_dram_parameter("bag", [1, LAYERS * HID], BF16, isOutput=False)
    p_biaspb = nc.declare_dram_parameter("biaspb", [128, 1 + LAYERS], F32, isOutput=False)
    p_bn = nc.declare_dram_parameter("bn", [128, 4], F32, isOutput=False)
    p_wfx = nc.declare_dram_parameter("wfx", [HID, OUT_DIM], F32, isOutput=False)
    p_wfu = nc.declare_dram_parameter("wfu", [HID, OUT_DIM], F32, isOutput=False)
    p_bfin = nc.declare_dram_parameter("bfin", [GPC, OUT_DIM], F32, isOutput=False)
    p_ones = nc.declare_dram_parameter("ones1", [1, 128], F32, isOutput=False)
    p_out = nc.declare_dram_parameter("out", [NPC, OUT_DIM], F32, isOutput=True)

    AG_RG = [list(range(NCORES))]
    nch = sched["nchunks"]
    calls = sched["calls"]
    chunk_index = sched["chunk_index"]

    with tile.TileContext(nc) as tc:
        with (
            tc.tile_pool(name="const", bufs=1) as const,
            tc.tile_pool(name="acts", bufs=2) as acts,
            tc.tile_pool(name="gbp", bufs=2) as gbp,
            tc.tile_pool(name="stp", bufs=4) as stp,
            tc.tile_pool(name="work", bufs=2) as work,
            tc.tile_pool(name="stats", bufs=1) as stats,
            tc.tile_pool(name="pscat", bufs=3, space="PSUM") as pscat,
            tc.tile_pool(name="pmisc", bufs=2, space="PSUM") as pmisc,
            tc.tile_pool(name="dram", bufs=2, space="DRAM") as dram,
        ):
            nc.gpsimd.load_library(mlp_library)

            # ---- load constants ----
            def cload(p, shape, dtype, tag):
                t = const.tile(shape, dtype, tag=tag)
                nc.sync.dma_start(t[:], p[:, :])
                return t

            xT0 = cload(p_xT0, [IN_DIM, NPC], F32, "xT0")
            idx_tiles = []
            for ci, call in enumerate(calls):
                if call["nchunks"] == 0:
                    idx_tiles.append(None)
                    continue
                it = const.tile([128, call["nchunks"] * 8], I16, tag=f"idx{ci}")
                nc.sync.dma_start(
                    it[:], p_idx[:, call["idx_col"]:call["idx_col"] + call["nchunks"] * 8])
                idx_tiles.append(it)
            dstrel_sb = cload(p_dstrel, list(p_dstrel.shape), F32, "dstrel")
            deg_sb = cload(p_deg, [1, NPC], BF16, "deg")
            iota_sb = cload(p_iota, [128, 128], BF16, "iota")
            wproj_sb = cload(p_wproj, [IN_DIM, HID], F32, "wproj")
            wl_sb = cload(p_wl, [HID, LAYERS * HID], F32, "wl")
            wa_sb = cload(p_wa, [HID, LAYERS * HID], F32, "wa")
            bag_sb = cload(p_bag, [1, LAYERS * HID], BF16, "bag")
            biaspb_sb = cload(p_biaspb, [128, 1 + LAYERS], F32, "biaspb")
            bn_sb = cload(p_bn, [128, 4], F32, "bn")
            wfx_sb = cload(p_wfx, [HID, OUT_DIM], F32, "wfx")
            wfu_sb = cload(p_wfu, [HID, OUT_DIM], F32, "wfu")
            bfin_sb = cload(p_bfin, [GPC, OUT_DIM], F32, "bfin")
            ones_sb = cload(p_ones, [1, 128], F32, "ones1")

            # ---- input projection + relu ----
            xT = acts.tile([HID, NPC], F32, tag="x")
            for j in range(NPC // 512):
                ps = pmisc.tile([128, 512], F32, tag="mm512")
                nc.tensor.matmul(ps[:], wproj_sb[:], xT0[:, j * 512:(j + 1) * 512],
                                 start=True, stop=True)
                nc.scalar.activation(xT[:, j * 512:(j + 1) * 512], ps[:],
                                     mybir.ActivationFunctionType.Relu,
                                     bias=biaspb_sb[:, 0:1])

            ug_parts = stats.tile([128, TPC], F32, tag="ug_parts")
            sx_parts = stats.tile([128, TPC], F32, tag="sx_parts")

            # ---- message-passing layers ----
            for li in range(nlayers):
                wa_i = wa_sb[:, li * HID:(li + 1) * HID]
                wl_i = wl_sb[:, li * HID:(li + 1) * HID]
                bag_i = bag_sb[:, li * HID:(li + 1) * HID]
                last = li == nlayers - 1

                # u_loc = x @ W_aggr  (node-major), -> DRAM bounce
                uloc_dram = dram.tile([NPC, HID], BF16, tag="uloc")
                uloc_d3 = uloc_dram.rearrange("(t p) h -> t p h", p=128)
                for t4 in range(TPC // 4):
                    ps = pmisc.tile([128, 512], F32, tag="mm512")
                    ub = work.tile([128, 512], BF16, tag="ubounce")
                    for q in range(4):
                        t = 4 * t4 + q
                        nc.tensor.matmul(ps[:, q * 128:(q + 1) * 128],
                                         xT[:, t * 128:(t + 1) * 128], wa_i,
                                         start=True, stop=True)
                    nc.vector.tensor_copy(ub[:], ps[:])
                    for q in range(4):
                        t = 4 * t4 + q
                        nc.sync.dma_start(uloc_d3[t], ub[:, q * 128:(q + 1) * 128])

                table = dram.tile([N, HID], BF16, tag="table")
                if not skip_ag:
                    nc.gpsimd.collective_compute(
                        "AllGather", mybir.AluOpType.bypass,
                        replica_groups=AG_RG,
                        ins=[uloc_dram[:].opt()],
                        outs=[table[:].opt()],
                    )
                else:
                    nc.sync.dma_start(table[0:NPC, :], uloc_dram[:])
                table_hi = dram.tile([HALF, HID], BF16, tag="table_hi")
                nc.sync.dma_start(table_hi[:], table[HALF:, :])

                xT_new = acts.tile([HID, NPC], F32, tag="x")

                # gather + scatter, per batch of TILE_BATCH dst tiles
                for b in range(TPC // TILE_BATCH):
                    cpair = calls[2 * b], calls[2 * b + 1]
                    gbufs = {}
                    for call in cpair:
                        if call["nchunks"] == 0:
                            continue
                        h = call["half"]
                        gb = gbp.tile([128, call["nchunks"], HID], BF16,
                                      tag=f"gb{h}")
                        src_view = table[0:HALF, :] if h == 0 else table_hi[:]
                        nidx = call["nchunks"] * 128
                        if skip_gather:
                            nc.vector.memset(gb[:], 0.0)
                        else:
                            nc.gpsimd.dma_gather(
                                gb[:], src_view, idx_tiles[2 * b + (0 if call is cpair[0] else 1)][:],
                                nidx, nidx, HID, single_packet=False,
                            )
                        gbufs[h] = (gb, call)

                    for t in range(b * TILE_BATCH, (b + 1) * TILE_BATCH):
                        pt = pscat.tile([128, HID], F32, tag="scat")
                        started = False
                        for h in (0, 1):
                            if h not in gbufs:
                                continue
                            gb, call = gbufs[h]
                            span = [s for s in call["spans"] if s[0] == t]
                            if not span:
                                continue
                            _, ck0, nk = span[0]
                            for k in range(nk):
                                st = stp.tile([128, 128], BF16, tag="s")
                                dk = chunk_index[(t, h, k)]
                                nc.vector.tensor_scalar(
                                    st[:], iota_sb[:], dstrel_sb[:, dk:dk + 1],
                                    None, mybir.AluOpType.is_equal)
                                nc.tensor.matmul(pt[:], gb[:, ck0 + k, :],
                                                 st[:],
                                                 start=not started, stop=False)
                                started = True
                        # degree * b_aggr (completes u for this tile)
                        nc.tensor.matmul(pt[:], bag_i,
                                         deg_sb[:, t * 128:(t + 1) * 128],
                                         start=not started, stop=last)
                        if not last:
                            # x_i = x @ W_layers accumulated on top
                            nc.tensor.matmul(pt[:], wl_i,
                                             xT[:, t * 128:(t + 1) * 128],
                                             start=False, stop=True)
                            nc.scalar.activation(
                                xT_new[:, t * 128:(t + 1) * 128], pt[:],
                                mybir.ActivationFunctionType.Relu,
                                bias=biaspb_sb[:, li + 1:li + 2])
                        else:
                            # u finished (group closed): u_g partials, then add
                            # x_i from a separate psum tile on DVE + relu.
                            nc.vector.tensor_reduce(
                                ug_parts[:, t:t + 1], pt[:],
                                mybir.AxisListType.X, mybir.AluOpType.add)
                            pxi = pmisc.tile([128, HID], F32, tag="mmfin")
                            nc.tensor.matmul(pxi[:], wl_i,
                                             xT[:, t * 128:(t + 1) * 128],
                                             start=True, stop=True)
                            xi_sb = work.tile([128, HID], F32, tag="xisb")
                            nc.scalar.activation(
                                xi_sb[:], pxi[:],
                                mybir.ActivationFunctionType.Copy)
                            tmp = work.tile([128, HID], F32, tag="xtmp")
                            nc.vector.scalar_tensor_tensor(
                                tmp[:], pt[:], biaspb_sb[:, li + 1:li + 2],
                                xi_sb[:], mybir.AluOpType.add,
                                mybir.AluOpType.add)
                            nc.vector.tensor_scalar(
                                xT_new[:, t * 128:(t + 1) * 128], tmp[:], 0.0,
                                None, mybir.AluOpType.max,
                                mybir.AluOpType.add,
                                accum_out=sx_parts[:, t:t + 1])
                xT = xT_new

            # ---- BatchNorm statistics ----
            ssx_parts = stats.tile([128, TPC], F32, tag="ssx_parts")
            for t in range(TPC):
                scr = work.tile([128, 128], BF16, tag="sqscr")
                nc.scalar.activation(scr[:], xT[:, t * 128:(t + 1) * 128],
                                     mybir.ActivationFunctionType.Square,
                                     accum_out=ssx_parts[:, t:t + 1])

            ug = stats.tile([128, GPC], F32, tag="ug")
            for g in range(GPC):
                nc.vector.tensor_reduce(ug[:, g:g + 1],
                                        ug_parts[:, g * 8:(g + 1) * 8],
                                        mybir.AxisListType.X, mybir.AluOpType.add)
            ugsq = stats.tile([128, GPC], F32, tag="ugsq")
            nc.vector.scalar_tensor_tensor(ugsq[:], ug[:], 0.0, ug[:],
                                           mybir.AluOpType.bypass,
                                           mybir.AluOpType.mult)
            pack = stats.tile([128, 4], F32, tag="pack")
            nc.vector.tensor_reduce(pack[:, 0:1], sx_parts[:],
                                    mybir.AxisListType.X, mybir.AluOpType.add)
            nc.vector.tensor_reduce(pack[:, 1:2], ssx_parts[:],
                                    mybir.AxisListType.X, mybir.AluOpType.add)
            nc.vector.tensor_reduce(pack[:, 2:3], ug[:],
                                    mybir.AxisListType.X, mybir.AluOpType.add)
            nc.vector.tensor_reduce(pack[:, 3:4], ugsq[:],
                                    mybir.AxisListType.X, mybir.AluOpType.add)
            # scale u-channel partials by nodes-per-graph
            nc.vector.tensor_scalar_mul(pack[:, 2:3], pack[:, 2:3], float(NPG))
            nc.vector.tensor_scalar_mul(pack[:, 3:4], pack[:, 3:4], float(NPG))

            ar_in = dram.tile([128, 4], F32, tag="ar_in")
            ar_out = dram.tile([128, 4], F32, tag="ar_out")
            nc.sync.dma_start(ar_in[:], pack[:])
            nc.gpsimd.collective_compute(
                "AllReduce", mybir.AluOpType.add,
                replica_groups=AG_RG,
                ins=[ar_in[:].opt()],
                outs=[ar_out[:].opt()],
            )
            gstats = stats.tile([128, 4], F32, tag="gstats")
            nc.sync.dma_start(gstats[:], ar_out[:])

            # mean/var -> scale/bias per channel, for x-half and u-half
            sb = {}
            for half_i, (s_col, q_col, g_col, b_col) in enumerate(
                    [(0, 1, 0, 1), (2, 3, 2, 3)]):
                mean = stats.tile([128, 1], F32, tag=f"mean{half_i}")
                var = stats.tile([128, 1], F32, tag=f"var{half_i}")
                rstd = stats.tile([128, 1], F32, tag=f"rstd{half_i}")
                scl = stats.tile([128, 1], F32, tag=f"scl{half_i}")
                bia = stats.tile([128, 1], F32, tag=f"bia{half_i}")
                nc.vector.tensor_scalar_mul(mean[:], gstats[:, s_col:s_col + 1], 1.0 / N)
                nc.vector.tensor_scalar_mul(var[:], gstats[:, q_col:q_col + 1], 1.0 / N)
                # var = E[x^2] - mean^2
                tmp = stats.tile([128, 1], F32, tag=f"tmp{half_i}")
                nc.vector.scalar_tensor_tensor(tmp[:], mean[:], 0.0, mean[:],
                                               mybir.AluOpType.bypass,
                                               mybir.AluOpType.mult)
                nc.vector.scalar_tensor_tensor(var[:], var[:], 0.0, tmp[:],
                                               mybir.AluOpType.bypass,
                                               mybir.AluOpType.subtract)
                std = stats.tile([128, 1], F32, tag=f"std{half_i}")
                nc.vector.tensor_scalar_add(var[:], var[:], EPS)
                nc.scalar.activation(std[:], var[:],
                                     mybir.ActivationFunctionType.Sqrt)
                nc.vector.reciprocal(rstd[:], std[:])
                nc.vector.scalar_tensor_tensor(scl[:], rstd[:], 0.0,
                                               bn_sb[:, g_col:g_col + 1],
                                               mybir.AluOpType.bypass,
                                               mybir.AluOpType.mult)
                nc.vector.scalar_tensor_tensor(tmp[:], mean[:], 0.0, scl[:],
                                               mybir.AluOpType.bypass,
                                               mybir.AluOpType.mult)
                nc.vector.scalar_tensor_tensor(bia[:], bn_sb[:, b_col:b_col + 1],
                                               0.0, tmp[:],
                                               mybir.AluOpType.bypass,
                                               mybir.AluOpType.subtract)
                sb[half_i] = (scl, bia)

            # normalize x -> bf16
            xTn = acts.tile([HID, NPC], F32, tag="x")
            for j in range(NPC // 512):
                nc.scalar.activation(xTn[:, j * 512:(j + 1) * 512],
                                     xT[:, j * 512:(j + 1) * 512],
                                     mybir.ActivationFunctionType.Identity,
                                     bias=sb[0][1][:], scale=sb[0][0][:])
            # normalize u_g -> bf16
            ugn = stats.tile([128, GPC], F32, tag="ugn")
            nc.scalar.activation(ugn[:], ug[:],
                                 mybir.ActivationFunctionType.Identity,
                                 bias=sb[1][1][:], scale=sb[1][0][:])

            # c_u[g,:] = u_gn[:,g] @ W_final[128:], + b_final
            cu_ps = pmisc.tile([GPC, OUT_DIM], F32, tag="mm512")
            nc.tensor.matmul(cu_ps[:], ugn[:], wfu_sb[:], start=True, stop=True)
            cub = stats.tile([GPC, OUT_DIM], F32, tag="cub")
            nc.vector.scalar_tensor_tensor(cub[:], cu_ps[:], 0.0, bfin_sb[:],
                                           mybir.AluOpType.bypass,
                                           mybir.AluOpType.add)
            cub16 = stats.tile([GPC, OUT_DIM], F32, tag="cub16")
            nc.vector.tensor_copy(cub16[:], cub[:])
            # move [8,2] -> single-partition row [1,16] (via DRAM) so the K=1
            # bias matmuls can read from partition 0
            cub_dram = dram.tile([GPC, OUT_DIM], F32, tag="cub_dram")
            nc.sync.dma_start(cub_dram[:], cub16[:])
            cubrow = stats.tile([1, GPC * OUT_DIM], F32, tag="cubrow")
            nc.sync.dma_start(
                cubrow[:], cub_dram[:].rearrange("g o -> (g o)")[None, :])

            # final matmul per tile + bias via K=1 trick
            for t in range(TPC):
                g = t // 8
                psf = pmisc.tile([128, OUT_DIM], F32, tag="mmfin")
                nc.tensor.matmul(psf[:], xTn[:, t * 128:(t + 1) * 128], wfx_sb[:],
                                 start=True, stop=False)
                nc.tensor.matmul(psf[:], ones_sb[:],
                                 cubrow[:, g * OUT_DIM:(g + 1) * OUT_DIM],
                                 start=False, stop=True)
                ot = work.tile([128, OUT_DIM], F32, tag="otile")
                nc.vector.tensor_copy(ot[:], psf[:])
                nc.sync.dma_start(p_out[t * 128:(t + 1) * 128, :], ot[:])

    nc.compile()
    return nc


def _bf16(a):
    return np.asarray(a, dtype=np.float32).astype(ml_dtypes.bfloat16)


def _make_in_maps(per_core, x, W_proj, b_proj, W_layers, b_layers, W_aggr,
                  b_aggr, bn_gamma, bn_beta, W_final, b_final):
    x = np.asarray(x, dtype=np.float32)
    iota_t = np.tile(np.arange(128, dtype=np.float32), (128, 1))
    shared = dict(
        iota=_bf16(iota_t),
        wproj=np.asarray(W_proj, np.float32),
        wl=np.concatenate(list(np.asarray(W_layers, np.float32)), axis=1),
        wa=np.concatenate(list(np.asarray(W_aggr, np.float32)), axis=1),
        bag=_bf16(np.asarray(b_aggr, np.float32).reshape(1, LAYERS * HID)),
        biaspb=np.concatenate(
            [np.asarray(b_proj, np.float32).reshape(128, 1),
             np.asarray(b_layers, np.float32).T], axis=1).astype(np.float32),
        bn=np.stack([np.asarray(bn_gamma, np.float32)[:128],
                     np.asarray(bn_beta, np.float32)[:128],
                     np.asarray(bn_gamma, np.float32)[128:],
                     np.asarray(bn_beta, np.float32)[128:]], axis=1).astype(np.float32),
        wfx=np.asarray(W_final, np.float32)[:HID],
        wfu=np.asarray(W_final, np.float32)[HID:],
        bfin=np.tile(np.asarray(b_final, np.float32).reshape(1, OUT_DIM),
                     (GPC, 1)).astype(np.float32),
        ones1=np.ones((1, 128), np.float32),
    )
    in_maps = []
    for c in range(NCORES):
        m = dict(shared)
        m["xT0"] = np.ascontiguousarray(x[c * NPC:(c + 1) * NPC].T)
        m["idx"] = per_core[c]["idx"]
        m["dstrel"] = per_core[c]["dstrel"]
        m["deg"] = per_core[c]["deg"]
        in_maps.append(m)
    return in_maps


def kernel(x, ei, n_nodes, W_proj, b_proj, W_layers, b_layers, W_aggr, b_aggr,
           bn_gamma, bn_beta, W_final, b_final):
    key = hash(np.asarray(ei).tobytes())
    if key not in _cache:
        sched, per_core = _host_prep(ei)
        nc = _build_nc(sched)
        _cache[key] = (nc, per_core)
    nc, per_core = _cache[key]
    in_maps = _make_in_maps(per_core, x, W_proj, b_proj, W_layers, b_layers,
                            W_aggr, b_aggr, bn_gamma, bn_beta, W_final, b_final)
    global _last_in_maps
    _last_in_maps = in_maps
    res = run_bass_kernel_spmd(nc, in_maps, core_ids=list(range(NCORES)))
    out = np.concatenate([res.results[c]["out"] for c in range(NCORES)], axis=0)
    return out.reshape(N // int(n_nodes), -1).astype(np.float32)


_last_in_maps = None

